# revision 26
# baseline (speedup 1.0000x reference)
"""Causal temporal attention kernel for 8 Trainium2 NeuronCores.

Reference computation (per batch b):
    qkv = x @ w_qkv + b_qkv ; split into q,k,v heads [H=16, Dh=64]
    q += pos_bias ; S = q k^T * Dh^-0.5 ; causal softmax ; out = S v
    y = concat_heads(out) @ w_out + b_out

Sharding: batch 2-way x head-group 4-way -> 8 cores. Core c = b*4 + g
computes heads 4g..4g+3 of batch b and returns the partial
y_part = concat(out_heads) @ w_out[rows of its heads]  ([T, DIM], bf16).
Host sums the 4 partials per batch and adds b_out.

v3 layout (same math as v2, restructured to close Tensor-engine gaps):
  * v_sb per-(head,chunk) stride widened 65 -> 128 with columns 64-127
    all-ones: the AV matmul's output rows 64-127 become the softmax
    denominator replicated across 64 partitions for free (matmul time
    is moving-row-bound, not output-partition-bound).
  * Norm: DVE copies the numerator + one denominator row off PSUM
    immediately (frees the o banks ~1.4us after the last AV), then the
    exact reciprocal runs partition-major [128,4] via a DRAM reshape
    round-trip and a DRAM-broadcast fans 1/d back to 64 partitions.
    (Rejected alternatives, all measured: DVE reciprocal on [64,512]
    costs 6.4ns/col = 3.3us on the critical path; reciprocal_approx_*
    custom-DVE ops return garbage on HW; ACT exp(-ln d) thrashes the
    activation table against the softmax Exp stream, ~1.3us per swap.)
  * Attention chunks run off-diagonal first: the diagonal k-chunks
    need kt/v of the slice projected in the PREVIOUS merged round,
    whose evacs land late; off-diag chunks depend on ancient slices.
  * AV trails exp by two chunks; the p=0 flush/norm units are stitched
    into p=1's first (QK-only) units so the PE never waits for the exp
    pipeline to refill or for the o-bank recycle at the p seam.
  * Input DMAs split across queues: weights on sync, xt slice 0 on
    gpsimd+scalar, slices 1-3 on gpsimd; y stores alternate
    gpsimd/sync. PE warm-up runs in the psO pool, which is untouched
    until the first attention round.
"""

import sys

sys.path.insert(0, "/opt/trn_rl_repo")

from contextlib import ExitStack

import numpy as np

import concourse.bacc as bacc
import concourse.tile as tile
from concourse import mybir
from concourse.bass_utils import run_bass_kernel_spmd

F32 = mybir.dt.float32
F32R = mybir.dt.float32r
BF16 = mybir.dt.bfloat16
EXP = mybir.ActivationFunctionType.Exp
LN = mybir.ActivationFunctionType.Ln

B, T, DIM = 2, 2048, 1024
HEADS, DH = 16, 64
HPC = 4              # heads per core
NCORES = 8
SCALE = DH ** -0.5
NSL = 4              # 512-token slices / q-tiles
KCH = T // 128       # 16 k-chunks of 128
VSTRIDE = KCH * 128  # per-head stride in v_sb
NARROW_EXP = True
AV_LAG = 2


def _merge(a, b):
    """Proportionally interleave two unit generators (lists of thunks)."""
    out = []
    ia = ib = 0
    while ia < len(a) or ib < len(b):
        if ib >= len(b) or (ia < len(a) and ia * (len(b) or 1) <= ib * (len(a) or 1)):
            out.append(a[ia]); ia += 1
        else:
            out.append(b[ib]); ib += 1
    return out


def _build_nc():
    nc = bacc.Bacc("TRN2", target_bir_lowering=False, debug=False,
                   num_devices=NCORES)
    xt_d = nc.dram_tensor("xt", [DIM, T], BF16, kind="ExternalInput").ap()
    wqk_d = nc.dram_tensor("wqk", [DIM, 512], BF16, kind="ExternalInput").ap()
    wv_d = nc.dram_tensor("wv", [DIM, HPC * DH], BF16, kind="ExternalInput").ap()
    qb_d = nc.dram_tensor("qbias", [128, 2], F32, kind="ExternalInput").ap()
    kb_d = nc.dram_tensor("kbias", [128, 2], F32, kind="ExternalInput").ap()
    bvb_d = nc.dram_tensor("bvb", [128, HPC * DH], F32, kind="ExternalInput").ap()
    wout_d = nc.dram_tensor("wout", [2, 128, DIM], BF16, kind="ExternalInput").ap()
    mask_d = nc.dram_tensor("masktri", [128, 128], BF16, kind="ExternalInput").ap()
    y_d = nc.dram_tensor("y", [T, DIM], BF16, kind="ExternalOutput").ap()
    rb_d = nc.dram_tensor("rbscratch", [2 * NSL * 2, 512], F32).ap()
    rb2_d = nc.dram_tensor("rbscratch2", [2 * NSL * 2, 512], F32).ap()

    with tile.TileContext(nc) as tc, ExitStack() as ctx:
        res = ctx.enter_context(tc.tile_pool(name="res", bufs=1))
        small = ctx.enter_context(tc.tile_pool(name="small", bufs=4))

        # ---- resident input tiles + DMA issue plan ----
        wqk_t, wv_t = [], []
        xt_t = {}
        for c in range(8):
            w = res.tile([128, 512], BF16, tag=f"wqk{c}", name=f"wqk{c}")
            wqk_t.append(w)
            w = res.tile([128, HPC * DH], BF16, tag=f"wv{c}", name=f"wv{c}")
            wv_t.append(w)
            for sl in range(NSL):
                t_ = res.tile([128, 512], BF16, tag=f"xt{c}_{sl}",
                              name=f"xt{c}_{sl}")
                xt_t[(c, sl)] = t_
        qb = res.tile([128, 2], F32, tag="qb")
        kb = res.tile([128, 2], F32, tag="kb")
        bvb = res.tile([128, HPC * DH], F32, tag="bvb")
        maskT = res.tile([128, 128], BF16, tag="maskT")
        wout_t = []
        for p in range(2):
            w = res.tile([128, DIM], BF16, tag=f"wout{p}", name=f"wout{p}")
            wout_t.append(w)

        # Startup feed at the PE's consumption rate (~0.74us per c-chunk
        # pair): even wqk on sync, odd wqk interleaved with the late xt
        # halves on scalar, xt c=0..3 on gpsimd. One queue alone delivers
        # a chunk only every ~0.61us and starves the first QT block.
        def _wqk(c, q):
            q.dma_start(wqk_t[c][:], wqk_d[c * 128:(c + 1) * 128, :])

        def _xt0(c, q):
            q.dma_start(xt_t[(c, 0)][:],
                        xt_d[c * 128:(c + 1) * 128, 0:512])

        for c in (0, 2, 4, 6):
            _wqk(c, nc.sync)
        for c in range(4):
            _xt0(c, nc.gpsimd)
        _wqk(1, nc.scalar)
        _wqk(3, nc.scalar)
        _xt0(4, nc.scalar)
        _xt0(5, nc.scalar)
        _wqk(5, nc.scalar)
        _xt0(6, nc.scalar)
        _wqk(7, nc.scalar)
        _xt0(7, nc.scalar)
        nc.sync.dma_start(qb[:], qb_d[:, :])
        nc.sync.dma_start(kb[:], kb_d[:, :])
        nc.sync.dma_start(bvb[:], bvb_d[:, :])
        nc.sync.dma_start(maskT[:], mask_d[:, :])
        for c in range(8):
            nc.sync.dma_start(wv_t[c][:],
                              wv_d[c * 128:(c + 1) * 128, :])
        for sl in range(1, NSL):
            for c in range(8):
                nc.gpsimd.dma_start(
                    xt_t[(c, sl)][:],
                    xt_d[c * 128:(c + 1) * 128,
                         sl * 512:(sl + 1) * 512])

        # ---- persistent compute tiles ----
        qt_sb, kt_sb, outT = [], [], []
        for p in range(2):
            qt_sb.append(res.tile([128, T], BF16, tag=f"qt{p}", name=f"qt{p}"))
            kt_sb.append(res.tile([128, T], BF16, tag=f"kt{p}", name=f"kt{p}"))
            outT.append(res.tile([128, T], BF16, tag=f"outT{p}", name=f"outT{p}"))
        v_sb = res.tile([128, HPC * VSTRIDE], BF16, tag="v_sb")

        warm = res.tile([1, 512], BF16, tag="warm")
        nc.gpsimd.memset(warm[:], 1.0)
        ones64 = res.tile([1, 64], BF16, tag="ones64")
        nc.gpsimd.memset(ones64[:], 1.0)

        # v_sb all-ones once (DVE, ~7us, hidden under the input-DMA window):
        # V evacs overwrite cols 0:64 of each (head, chunk) block; cols
        # 64:128 stay 1.0, so the AV matmul emits the softmax denominator
        # replicated in PSUM rows 64:128 for free.
        nc.vector.memset(v_sb[:], 1.0)
        for p in range(2):
            nc.sync.dma_start(wout_t[p][:], wout_d[p])

        # ---- main pools: exactly 8 PSUM banks ----
        psP = ctx.enter_context(tc.tile_pool(name="psP", bufs=2, space="PSUM"))
        psS = ctx.enter_context(tc.tile_pool(name="psS", bufs=2, space="PSUM"))
        psO = ctx.enter_context(tc.tile_pool(name="psO", bufs=2, space="PSUM"))
        ptp = ctx.enter_context(tc.tile_pool(name="ptp", bufs=8))
        ysp = ctx.enter_context(tc.tile_pool(name="ysp", bufs=3))
        nsm = ctx.enter_context(tc.tile_pool(name="nsm", bufs=4))
        rbp = ctx.enter_context(tc.tile_pool(name="rbp", bufs=3))

        # ---- PE warm-up burst while the first DMAs stream in ----
        # psO is untouched until the first attention round (~20us in), so
        # warm-up tiles there never gate the first projection matmuls.
        for i in range(6):
            wp = psO.tile([64, 512], F32, tag="o", name=f"warm{i}")
            nc.tensor.matmul(wp[:], ones64[:], warm[:], start=True,
                             stop=True)

        def proj_units(sl):
            """QKV projection for token slice sl -> list of thunks."""
            units = []
            # QT/KT: groups g = qk*2+p; two [128,512] banks at a time.
            for blk in range(2):
                state = {}

                def open_blk(state=state, blk=blk, sl=sl):
                    state["ps"] = [
                        psP.tile([128, 512], F32, tag="proj",
                                 name=f"qkps{sl}{blk}{gi}")
                        for gi in range(2)]

                for cp in range(4):
                    def run(cp=cp, blk=blk, sl=sl, state=state, ob=open_blk):
                        if cp == 0:
                            ob()
                        for c in (2 * cp, 2 * cp + 1):
                            for gi in range(2):
                                g = blk * 2 + gi
                                qk, p = divmod(g, 2)
                                col = (qk * 2 + p) * 128
                                nc.tensor.matmul(
                                    state["ps"][gi][:],
                                    wqk_t[c][:, col:col + 128],
                                    xt_t[(c, sl)][:],
                                    start=(c == 0), stop=(c == 7))
                    units.append(run)

                def evac(blk=blk, sl=sl, state=state):
                    for gi in range(2):
                        g = blk * 2 + gi
                        qk, p = divmod(g, 2)
                        dst = (qt_sb if qk == 0 else kt_sb)[p]
                        bias = (qb if qk == 0 else kb)[:, p:p + 1]
                        nc.vector.tensor_add(
                            dst[:, sl * 512:(sl + 1) * 512], state["ps"][gi][:],
                            bias.to_broadcast((128, 512)))
                units.append(evac)
            # V: m-chunks, two [128,256] outputs per block in proj banks.
            for blk in range(2):
                state = {}

                def open_blk(state=state, blk=blk, sl=sl):
                    state["ps"] = [
                        psP.tile([128, 512], F32, tag="proj",
                                 name=f"vps{sl}{blk}{mi}")
                        for mi in range(2)]

                for cp in range(4):
                    def run(cp=cp, blk=blk, sl=sl, state=state, ob=open_blk):
                        if cp == 0:
                            ob()
                        for c in (2 * cp, 2 * cp + 1):
                            for mi in range(2):
                                ml = blk * 2 + mi
                                nc.tensor.matmul(
                                    state["ps"][mi][:, 0:HPC * DH],
                                    xt_t[(c, sl)][:, ml * 128:(ml + 1) * 128],
                                    wv_t[c][:],
                                    start=(c == 0), stop=(c == 7))
                    units.append(run)

                def evac(blk=blk, sl=sl, state=state):
                    for mi in range(2):
                        m = sl * 4 + blk * 2 + mi
                        vv = v_sb.rearrange("a (h m f) -> a h m f",
                                            h=HPC, m=KCH)
                        nc.vector.tensor_add(
                            vv[:, :, m, 0:64],
                            state["ps"][mi][:, 0:HPC * DH].rearrange(
                                "a (h f) -> a h f", h=HPC),
                            bvb[:, :].rearrange("a (h f) -> a h f", h=HPC))
                units.append(evac)
            return units

        def attn_units(qi):
            """Attention + output projection for q-tile qi."""
            units = []
            qs = slice(qi * 512, (qi + 1) * 512)
            # off-diagonal chunks first (their kt/v are from old slices);
            # diagonal chunks last (their kt/v evac lands mid-round).
            js = list(range(0, 4 * qi)) + list(range(4 * qi, 4 * qi + 4))
            per_p = []
            for p in range(2):
                state = {"pend": [], "issued": 0, "o": None}

                def chunk(ji, j, p=p, qi=qi, state=state, js=js):
                    if j is not None:
                        jl = j - 4 * qi
                        w0 = 128 * jl if jl >= 0 else 0
                        st = psS.tile([128, 1024], F32, tag="st",
                                      name=f"st{qi}{p}{j}")
                        for hl in range(2):
                            rows = slice(hl * 64, hl * 64 + 64)
                            nc.tensor.matmul(
                                st[:, hl * 512 + w0:(hl + 1) * 512],
                                kt_sb[p][rows, j * 128:(j + 1) * 128],
                                qt_sb[p][rows, qi * 512 + w0:(qi + 1) * 512],
                                start=True, stop=True)
                            if jl >= 0:
                                nc.vector.tensor_add(
                                    st[:, hl * 512 + w0:hl * 512 + w0 + 128],
                                    st[:, hl * 512 + w0:hl * 512 + w0 + 128],
                                    maskT[:])
                        pt = ptp.tile([128, 1024], BF16, tag="pt",
                                      name=f"pt{qi}{p}{j}")
                        if w0 and NARROW_EXP:
                            stv = st.rearrange("a (h q) -> a h q", h=2)
                            ptv = pt.rearrange("a (h q) -> a h q", h=2)
                            nc.scalar.activation(ptv[:, :, w0:512],
                                                 stv[:, :, w0:512], EXP,
                                                 scale=SCALE)
                        else:
                            nc.scalar.activation(pt[:], st[:], EXP,
                                                 scale=SCALE)
                        state["pend"].append((j, pt, w0))
                    if (len(state["pend"]) > AV_LAG
                            or (j is None and state["pend"])):
                        if state["issued"] == 0:
                            # alloc at first AV issue, not unit 0: lets the
                            # seam stitch emit the previous p's norm (which
                            # releases these banks) before this runs.
                            state["o"] = [
                                psO.tile([128, 512], F32, tag="o",
                                         name=f"o{qi}{p}{hl}")
                                for hl in range(2)]
                        jp, ptq, w0p = state["pend"].pop(0)
                        for hl in range(2):
                            h = 2 * p + hl
                            off = (h * KCH + jp) * 128
                            nc.tensor.matmul(
                                state["o"][hl][:, w0p:512],
                                v_sb[:, off:off + 128],
                                ptq[:, hl * 512 + w0p:(hl + 1) * 512],
                                start=(state["issued"] == 0),
                                stop=(state["issued"] == len(js) - 1))
                        state["issued"] += 1

                cu = [lambda ji=ji, j=j, fn=chunk: fn(ji, j)
                      for ji, j in enumerate(js + [None] * AV_LAG)]

                def norm(p=p, qi=qi, state=state):
                    # DVE copies numerator + denominator row off PSUM first
                    # (releases the o banks ~1.4us after the last AV), then
                    # the exact reciprocal runs partition-major [128,4] via
                    # a DRAM reshape round-trip (DVE reciprocal costs
                    # 6.4ns/free-col, so [64,512] directly is 3.3us; ACT
                    # ln/exp thrashes the activation table; approx_fast is
                    # wrong on HW), and a DRAM-broadcast DMA fans 1/d back
                    # to 64 partitions for the multiply.
                    for hl in range(2):
                        rows = slice(hl * 64, hl * 64 + 64)
                        o = state["o"][hl]
                        idx = (qi * 2 + p) * 2 + hl
                        # hl=0 chain on sync, hl=1 on gpsimd: the two
                        # 4-DMA round-trips run in parallel instead of
                        # serializing on one queue (the exposed final-tile
                        # norm was 11.8us with both chains on sync).
                        dq = nc.sync if hl == 0 else nc.gpsimd
                        s_sb = nsm.tile([1, 512], F32, tag="s_sb",
                                        name=f"s{qi}{p}{hl}")
                        nc.vector.tensor_copy(s_sb[:], o[64:65, :])
                        nm_sb = nsm.tile([64, 512], BF16, tag="nm_sb",
                                         name=f"nm{qi}{p}{hl}")
                        nc.vector.tensor_copy(nm_sb[:], o[0:64, :])
                        dq.dma_start(rb_d[idx:idx + 1, :], s_sb[0:1, :])
                        s_pd = nsm.tile([128, 4], F32, tag="s_pd")
                        dq.dma_start(
                            s_pd[:, :],
                            rb_d[idx:idx + 1, :].rearrange(
                                "o (p f) -> (o p) f", p=128))
                        r_pd = nsm.tile([128, 4], F32, tag="r_pd")
                        nc.vector.reciprocal(r_pd[:], s_pd[:])
                        dq.dma_start(
                            rb2_d[idx:idx + 1, :].rearrange(
                                "o (p f) -> (o p) f", p=128),
                            r_pd[:, :])
                        rb = rbp.tile([64, 512], F32, tag="rb",
                                      name=f"rb{qi}{p}{hl}")
                        dq.dma_start(
                            rb[:, :],
                            rb2_d[idx:idx + 1, :].to_broadcast((64, 512)))
                        nc.vector.tensor_mul(
                            outT[p][rows, qi * 512:(qi + 1) * 512],
                            nm_sb[:], rb[:])
                per_p.append((cu, norm))

            # Stitch the p=0 flush/norm units into p=1's first (QK-only)
            # units: p1's exps start while p0's last AVs drain, and p0's
            # norm (which frees the o banks) lands before p1's first AV.
            cu0, n0 = per_p[0]
            cu1, n1 = per_p[1]
            units += cu0[:-AV_LAG]
            units += [cu1[0], cu0[-2], cu1[1], cu0[-1], n0]
            units += cu1[2:]
            units.append(n1)
            for qc in range(4 * qi, 4 * qi + 4):
                def yproj(qc=qc):
                    qcs = slice(qc * 128, (qc + 1) * 128)
                    y_ps = [psP.tile([128, 512], F32, tag="proj",
                                     name=f"y{qc}{ct}") for ct in range(2)]
                    for p in range(2):
                        for ct in range(2):
                            nc.tensor.matmul(
                                y_ps[ct][:], outT[p][:, qcs],
                                wout_t[p][:, ct * 512:(ct + 1) * 512],
                                start=(p == 0), stop=(p == 1))
                    for ct in range(2):
                        y_sb = ysp.tile([128, 512], BF16, tag="ysb",
                                        name=f"ysb{qc}{ct}")
                        nc.vector.tensor_copy(y_sb[:], y_ps[ct][:])
                        q_eng = nc.gpsimd if ct == 0 else nc.sync
                        q_eng.dma_start(y_d[qcs, ct * 512:(ct + 1) * 512],
                                        y_sb[:])
                units.append(yproj)
            return units

        # ---- emit rounds ----
        rounds = [proj_units(0)]
        for qi in range(NSL):
            a = attn_units(qi)
            b = proj_units(qi + 1) if qi + 1 < NSL else []
            rounds.append(_merge(a, b))
        for rr in rounds:
            for u in rr:
                u()

    nc.compile()
    return nc


_NC = None


def _get_nc():
    global _NC
    if _NC is None:
        _NC = _build_nc()
    return _NC


def _host_shards(x, w_qkv, b_qkv, w_out, b_out, pos_bias):
    import ml_dtypes
    x = np.asarray(x, dtype=np.float32)
    w_qkv = np.asarray(w_qkv, dtype=np.float32)
    b_qkv = np.asarray(b_qkv, dtype=np.float32)
    w_out = np.asarray(w_out, dtype=np.float32)
    pos_bias = np.asarray(pos_bias, dtype=np.float32).reshape(HEADS, DH)

    wq, wk, wv = w_qkv[:, :DIM], w_qkv[:, DIM:2 * DIM], w_qkv[:, 2 * DIM:]
    bq, bk, bv = b_qkv[:DIM], b_qkv[DIM:2 * DIM], b_qkv[2 * DIM:]

    dk = np.arange(128)[:, None]
    dq = np.arange(128)[None, :]
    masktri = np.where(dk <= dq, 0.0, -240.0).astype(ml_dtypes.bfloat16)

    maps = []
    for core in range(NCORES):
        b, g = divmod(core, HPC)
        h0 = HPC * g
        cols = slice(h0 * DH, (h0 + HPC) * DH)          # 256 head dims
        pair_cols = [slice((h0 + 2 * p) * DH, (h0 + 2 * p + 2) * DH)
                     for p in range(2)]
        wqk_c = np.concatenate(
            [wq[:, pair_cols[0]], wq[:, pair_cols[1]],
             wk[:, pair_cols[0]], wk[:, pair_cols[1]]], axis=1)
        qbias = np.stack(
            [bq[pair_cols[p]]
             + pos_bias[h0 + 2 * p:h0 + 2 * p + 2].reshape(-1)
             for p in range(2)], axis=1)
        kbias = np.stack([bk[pair_cols[p]] for p in range(2)], axis=1)
        bvb = np.broadcast_to(bv[cols], (128, HPC * DH))
        wout_c = np.stack([w_out[pair_cols[p], :] for p in range(2)])
        maps.append({
            "xt": np.ascontiguousarray(x[b].T).astype(ml_dtypes.bfloat16),
            "wqk": np.ascontiguousarray(wqk_c).astype(ml_dtypes.bfloat16),
            "wv": np.ascontiguousarray(wv[:, cols]).astype(ml_dtypes.bfloat16),
            "qbias": np.ascontiguousarray(qbias),
            "kbias": np.ascontiguousarray(kbias),
            "bvb": np.ascontiguousarray(bvb),
            "wout": np.ascontiguousarray(wout_c).astype(ml_dtypes.bfloat16),
            "masktri": masktri,
        })
    return maps


def kernel(x, w_qkv, b_qkv, w_out, b_out, pos_bias, _trace=False):
    nc = _get_nc()
    in_maps = _host_shards(x, w_qkv, b_qkv, w_out, b_out, pos_bias)
    res = run_bass_kernel_spmd(nc, in_maps, list(range(NCORES)),
                               trace=_trace)
    b_out = np.asarray(b_out, dtype=np.float32)
    y = np.empty((B, T, DIM), dtype=np.float32)
    for b in range(B):
        acc = res.results[b * HPC]["y"].astype(np.float32)
        for g in range(1, HPC):
            acc = acc + res.results[b * HPC + g]["y"].astype(np.float32)
        y[b] = acc + b_out
    if _trace:
        kernel._last_results = res
    return y


# revision 27
# speedup vs baseline: 1.0911x; 1.0911x over previous
"""Causal temporal attention kernel for 8 Trainium2 NeuronCores.

Reference computation (per batch b):
    qkv = x @ w_qkv + b_qkv ; split into q,k,v heads [H=16, Dh=64]
    q += pos_bias ; S = q k^T * Dh^-0.5 ; causal softmax ; out = S v
    y = concat_heads(out) @ w_out + b_out

Sharding: batch 2-way x head-group 4-way -> 8 cores. Core c = b*4 + g
computes heads 4g..4g+3 of batch b and returns the partial
y_part = concat(out_heads) @ w_out[rows of its heads]  ([T, DIM], bf16).
Host sums the 4 partials per batch and adds b_out.

v3 layout (same math as v2, restructured to close Tensor-engine gaps):
  * v_sb per-(head,chunk) stride widened 65 -> 128 with columns 64-127
    all-ones: the AV matmul's output rows 64-127 become the softmax
    denominator replicated across 64 partitions for free (matmul time
    is moving-row-bound, not output-partition-bound).
  * Norm: DVE copies the numerator + one denominator row off PSUM
    immediately (frees the o banks ~1.4us after the last AV), then the
    exact reciprocal runs partition-major [128,4] via a DRAM reshape
    round-trip and a DRAM-broadcast fans 1/d back to 64 partitions.
    (Rejected alternatives, all measured: DVE reciprocal on [64,512]
    costs 6.4ns/col = 3.3us on the critical path; reciprocal_approx_*
    custom-DVE ops return garbage on HW; ACT exp(-ln d) thrashes the
    activation table against the softmax Exp stream, ~1.3us per swap.)
  * Attention chunks run off-diagonal first: the diagonal k-chunks
    need kt/v of the slice projected in the PREVIOUS merged round,
    whose evacs land late; off-diag chunks depend on ancient slices.
  * AV trails exp by two chunks; the p=0 flush/norm units are stitched
    into p=1's first (QK-only) units so the PE never waits for the exp
    pipeline to refill or for the o-bank recycle at the p seam.
  * Input DMAs split across queues: weights on sync, xt slice 0 on
    gpsimd+scalar, slices 1-3 on gpsimd; y stores alternate
    gpsimd/sync. PE warm-up runs in the psO pool, which is untouched
    until the first attention round.
"""

import sys

sys.path.insert(0, "/opt/trn_rl_repo")

from contextlib import ExitStack

import numpy as np

import concourse.bacc as bacc
import concourse.tile as tile
from concourse import mybir
from concourse.bass_utils import run_bass_kernel_spmd

F32 = mybir.dt.float32
F32R = mybir.dt.float32r
BF16 = mybir.dt.bfloat16
EXP = mybir.ActivationFunctionType.Exp
LN = mybir.ActivationFunctionType.Ln

B, T, DIM = 2, 2048, 1024
HEADS, DH = 16, 64
HPC = 4              # heads per core
NCORES = 8
SCALE = DH ** -0.5
NSL = 4              # 512-token slices / q-tiles
KCH = T // 128       # 16 k-chunks of 128
VSTRIDE = KCH * 128  # per-head stride in v_sb
NARROW_EXP = True
AV_LAG = 2


def _merge(a, b):
    """Proportionally interleave two unit generators (lists of thunks)."""
    out = []
    ia = ib = 0
    while ia < len(a) or ib < len(b):
        if ib >= len(b) or (ia < len(a) and ia * (len(b) or 1) <= ib * (len(a) or 1)):
            out.append(a[ia]); ia += 1
        else:
            out.append(b[ib]); ib += 1
    return out


def _build_nc():
    nc = bacc.Bacc("TRN2", target_bir_lowering=False, debug=False,
                   num_devices=NCORES)
    xt_d = nc.dram_tensor("xt", [DIM, T], BF16, kind="ExternalInput").ap()
    wqk_d = nc.dram_tensor("wqk", [DIM, 512], BF16, kind="ExternalInput").ap()
    wv_d = nc.dram_tensor("wv", [DIM, HPC * DH], BF16, kind="ExternalInput").ap()
    qb_d = nc.dram_tensor("qbias", [128, 2], F32, kind="ExternalInput").ap()
    kb_d = nc.dram_tensor("kbias", [128, 2], F32, kind="ExternalInput").ap()
    bvb_d = nc.dram_tensor("bvb", [128, HPC * DH], F32, kind="ExternalInput").ap()
    wout_d = nc.dram_tensor("wout", [2, 128, DIM], BF16, kind="ExternalInput").ap()
    mask_d = nc.dram_tensor("masktri", [128, 128], BF16, kind="ExternalInput").ap()
    y_d = nc.dram_tensor("y", [T, DIM], BF16, kind="ExternalOutput").ap()
    rb_d = nc.dram_tensor("rbscratch", [2 * NSL * 2, 512], F32).ap()
    rb2_d = nc.dram_tensor("rbscratch2", [2 * NSL * 2, 512], F32).ap()

    with tile.TileContext(nc) as tc, ExitStack() as ctx:
        res = ctx.enter_context(tc.tile_pool(name="res", bufs=1))
        small = ctx.enter_context(tc.tile_pool(name="small", bufs=4))

        # ---- resident input tiles + DMA issue plan ----
        wqk_t, wv_t = [], []
        xt_t = {}
        for c in range(8):
            w = res.tile([128, 512], BF16, tag=f"wqk{c}", name=f"wqk{c}")
            wqk_t.append(w)
            w = res.tile([128, HPC * DH], BF16, tag=f"wv{c}", name=f"wv{c}")
            wv_t.append(w)
            for sl in range(NSL):
                t_ = res.tile([128, 512], BF16, tag=f"xt{c}_{sl}",
                              name=f"xt{c}_{sl}")
                xt_t[(c, sl)] = t_
        qb = res.tile([128, 2], F32, tag="qb")
        kb = res.tile([128, 2], F32, tag="kb")
        bvb = res.tile([128, HPC * DH], F32, tag="bvb")
        maskT = res.tile([128, 128], BF16, tag="maskT")
        wout_t = []
        for p in range(2):
            w = res.tile([128, DIM], BF16, tag=f"wout{p}", name=f"wout{p}")
            wout_t.append(w)

        # sync queue: wqk first (gates the projection), small tensors,
        # wv, then wout. gpsimd queue: xt slice 0 first, then slices 1-3.
        for c in range(8):
            nc.sync.dma_start(wqk_t[c][:],
                              wqk_d[c * 128:(c + 1) * 128, :])
            xq = nc.gpsimd if c < 4 else nc.scalar
            xq.dma_start(xt_t[(c, 0)][:],
                         xt_d[c * 128:(c + 1) * 128, 0:512])
        nc.sync.dma_start(qb[:], qb_d[:, :])
        nc.sync.dma_start(kb[:], kb_d[:, :])
        nc.sync.dma_start(bvb[:], bvb_d[:, :])
        nc.sync.dma_start(maskT[:], mask_d[:, :])
        for c in range(8):
            nc.sync.dma_start(wv_t[c][:],
                              wv_d[c * 128:(c + 1) * 128, :])
        for sl in range(1, NSL):
            for c in range(8):
                nc.gpsimd.dma_start(
                    xt_t[(c, sl)][:],
                    xt_d[c * 128:(c + 1) * 128,
                         sl * 512:(sl + 1) * 512])

        # ---- persistent compute tiles ----
        qt_sb, kt_sb, outT = [], [], []
        for p in range(2):
            qt_sb.append(res.tile([128, T], BF16, tag=f"qt{p}", name=f"qt{p}"))
            kt_sb.append(res.tile([128, T], BF16, tag=f"kt{p}", name=f"kt{p}"))
            outT.append(res.tile([128, T], BF16, tag=f"outT{p}", name=f"outT{p}"))
        v_sb = res.tile([128, HPC * VSTRIDE], BF16, tag="v_sb")

        warm = res.tile([1, 512], BF16, tag="warm")
        nc.gpsimd.memset(warm[:], 1.0)
        ones64 = res.tile([1, 64], BF16, tag="ones64")
        nc.gpsimd.memset(ones64[:], 1.0)

        # v_sb all-ones once (DVE, ~7us, hidden under the input-DMA window):
        # V evacs overwrite cols 0:64 of each (head, chunk) block; cols
        # 64:128 stay 1.0, so the AV matmul emits the softmax denominator
        # replicated in PSUM rows 64:128 for free.
        nc.vector.memset(v_sb[:], 1.0)
        for p in range(2):
            nc.sync.dma_start(wout_t[p][:], wout_d[p])

        # ---- main pools: exactly 8 PSUM banks ----
        psP = ctx.enter_context(tc.tile_pool(name="psP", bufs=2, space="PSUM"))
        psS = ctx.enter_context(tc.tile_pool(name="psS", bufs=2, space="PSUM"))
        psO = ctx.enter_context(tc.tile_pool(name="psO", bufs=2, space="PSUM"))
        ptp = ctx.enter_context(tc.tile_pool(name="ptp", bufs=8))
        ysp = ctx.enter_context(tc.tile_pool(name="ysp", bufs=3))
        nsm = ctx.enter_context(tc.tile_pool(name="nsm", bufs=4))
        rbp = ctx.enter_context(tc.tile_pool(name="rbp", bufs=3))

        # ---- PE warm-up burst while the first DMAs stream in ----
        # psO is untouched until the first attention round (~20us in), so
        # warm-up tiles there never gate the first projection matmuls.
        for i in range(6):
            wp = psO.tile([64, 512], F32, tag="o", name=f"warm{i}")
            nc.tensor.matmul(wp[:], ones64[:], warm[:], start=True,
                             stop=True)

        def proj_units(sl):
            """QKV projection for token slice sl -> list of thunks."""
            units = []
            # QT/KT: groups g = qk*2+p; two [128,512] banks at a time.
            for blk in range(2):
                state = {}

                def open_blk(state=state, blk=blk, sl=sl):
                    state["ps"] = [
                        psP.tile([128, 512], F32, tag="proj",
                                 name=f"qkps{sl}{blk}{gi}")
                        for gi in range(2)]

                for cp in range(4):
                    def run(cp=cp, blk=blk, sl=sl, state=state, ob=open_blk):
                        if cp == 0:
                            ob()
                        for c in (2 * cp, 2 * cp + 1):
                            for gi in range(2):
                                g = blk * 2 + gi
                                qk, p = divmod(g, 2)
                                col = (qk * 2 + p) * 128
                                nc.tensor.matmul(
                                    state["ps"][gi][:],
                                    wqk_t[c][:, col:col + 128],
                                    xt_t[(c, sl)][:],
                                    start=(c == 0), stop=(c == 7))
                    units.append(run)

                def evac(blk=blk, sl=sl, state=state):
                    for gi in range(2):
                        g = blk * 2 + gi
                        qk, p = divmod(g, 2)
                        dst = (qt_sb if qk == 0 else kt_sb)[p]
                        bias = (qb if qk == 0 else kb)[:, p:p + 1]
                        nc.vector.tensor_add(
                            dst[:, sl * 512:(sl + 1) * 512], state["ps"][gi][:],
                            bias.to_broadcast((128, 512)))
                units.append(evac)
            # V: m-chunks, two [128,256] outputs per block in proj banks.
            for blk in range(2):
                state = {}

                def open_blk(state=state, blk=blk, sl=sl):
                    state["ps"] = [
                        psP.tile([128, 512], F32, tag="proj",
                                 name=f"vps{sl}{blk}{mi}")
                        for mi in range(2)]

                for cp in range(4):
                    def run(cp=cp, blk=blk, sl=sl, state=state, ob=open_blk):
                        if cp == 0:
                            ob()
                        for c in (2 * cp, 2 * cp + 1):
                            for mi in range(2):
                                ml = blk * 2 + mi
                                nc.tensor.matmul(
                                    state["ps"][mi][:, 0:HPC * DH],
                                    xt_t[(c, sl)][:, ml * 128:(ml + 1) * 128],
                                    wv_t[c][:],
                                    start=(c == 0), stop=(c == 7))
                    units.append(run)

                def evac(blk=blk, sl=sl, state=state):
                    for mi in range(2):
                        m = sl * 4 + blk * 2 + mi
                        vv = v_sb.rearrange("a (h m f) -> a h m f",
                                            h=HPC, m=KCH)
                        nc.vector.tensor_add(
                            vv[:, :, m, 0:64],
                            state["ps"][mi][:, 0:HPC * DH].rearrange(
                                "a (h f) -> a h f", h=HPC),
                            bvb[:, :].rearrange("a (h f) -> a h f", h=HPC))
                units.append(evac)
            return units

        def attn_units(qi):
            """Attention + output projection for q-tile qi."""
            units = []
            qs = slice(qi * 512, (qi + 1) * 512)
            # off-diagonal chunks first (their kt/v are from old slices);
            # diagonal chunks last (their kt/v evac lands mid-round).
            js = list(range(0, 4 * qi)) + list(range(4 * qi, 4 * qi + 4))
            per_p = []
            for p in range(2):
                state = {"pend": [], "issued": 0, "o": None}

                def chunk(ji, j, p=p, qi=qi, state=state, js=js):
                    if j is not None:
                        jl = j - 4 * qi
                        w0 = 128 * jl if jl >= 0 else 0
                        st = psS.tile([128, 1024], F32, tag="st",
                                      name=f"st{qi}{p}{j}")
                        for hl in range(2):
                            rows = slice(hl * 64, hl * 64 + 64)
                            nc.tensor.matmul(
                                st[:, hl * 512 + w0:(hl + 1) * 512],
                                kt_sb[p][rows, j * 128:(j + 1) * 128],
                                qt_sb[p][rows, qi * 512 + w0:(qi + 1) * 512],
                                start=True, stop=True)
                            if jl >= 0:
                                nc.vector.tensor_add(
                                    st[:, hl * 512 + w0:hl * 512 + w0 + 128],
                                    st[:, hl * 512 + w0:hl * 512 + w0 + 128],
                                    maskT[:])
                        pt = ptp.tile([128, 1024], BF16, tag="pt",
                                      name=f"pt{qi}{p}{j}")
                        if w0 and NARROW_EXP:
                            stv = st.rearrange("a (h q) -> a h q", h=2)
                            ptv = pt.rearrange("a (h q) -> a h q", h=2)
                            nc.scalar.activation(ptv[:, :, w0:512],
                                                 stv[:, :, w0:512], EXP,
                                                 scale=SCALE)
                        else:
                            nc.scalar.activation(pt[:], st[:], EXP,
                                                 scale=SCALE)
                        state["pend"].append((j, pt, w0))
                    if (len(state["pend"]) > AV_LAG
                            or (j is None and state["pend"])):
                        if state["issued"] == 0:
                            # alloc at first AV issue, not unit 0: lets the
                            # seam stitch emit the previous p's norm (which
                            # releases these banks) before this runs.
                            state["o"] = [
                                psO.tile([128, 512], F32, tag="o",
                                         name=f"o{qi}{p}{hl}")
                                for hl in range(2)]
                        jp, ptq, w0p = state["pend"].pop(0)
                        for hl in range(2):
                            h = 2 * p + hl
                            off = (h * KCH + jp) * 128
                            nc.tensor.matmul(
                                state["o"][hl][:, w0p:512],
                                v_sb[:, off:off + 128],
                                ptq[:, hl * 512 + w0p:(hl + 1) * 512],
                                start=(state["issued"] == 0),
                                stop=(state["issued"] == len(js) - 1))
                        state["issued"] += 1

                cu = [lambda ji=ji, j=j, fn=chunk: fn(ji, j)
                      for ji, j in enumerate(js + [None] * AV_LAG)]

                def norm(p=p, qi=qi, state=state):
                    # DVE copies numerator + denominator row off PSUM first
                    # (releases the o banks ~1.4us after the last AV), then
                    # the exact reciprocal runs partition-major [128,4] via
                    # a DRAM reshape round-trip (DVE reciprocal costs
                    # 6.4ns/free-col, so [64,512] directly is 3.3us; ACT
                    # ln/exp thrashes the activation table; approx_fast is
                    # wrong on HW), and a DRAM-broadcast DMA fans 1/d back
                    # to 64 partitions for the multiply.
                    for hl in range(2):
                        rows = slice(hl * 64, hl * 64 + 64)
                        o = state["o"][hl]
                        idx = (qi * 2 + p) * 2 + hl
                        s_sb = nsm.tile([1, 512], F32, tag="s_sb",
                                        name=f"s{qi}{p}{hl}")
                        nc.vector.tensor_copy(s_sb[:], o[64:65, :])
                        nm_sb = nsm.tile([64, 512], BF16, tag="nm_sb",
                                         name=f"nm{qi}{p}{hl}")
                        nc.vector.tensor_copy(nm_sb[:], o[0:64, :])
                        nc.sync.dma_start(rb_d[idx:idx + 1, :], s_sb[0:1, :])
                        s_pd = nsm.tile([128, 4], F32, tag="s_pd")
                        nc.sync.dma_start(
                            s_pd[:, :],
                            rb_d[idx:idx + 1, :].rearrange(
                                "o (p f) -> (o p) f", p=128))
                        r_pd = nsm.tile([128, 4], F32, tag="r_pd")
                        nc.vector.reciprocal(r_pd[:], s_pd[:])
                        nc.sync.dma_start(
                            rb2_d[idx:idx + 1, :].rearrange(
                                "o (p f) -> (o p) f", p=128),
                            r_pd[:, :])
                        rb = rbp.tile([64, 512], F32, tag="rb",
                                      name=f"rb{qi}{p}{hl}")
                        nc.sync.dma_start(
                            rb[:, :],
                            rb2_d[idx:idx + 1, :].to_broadcast((64, 512)))
                        nc.vector.tensor_mul(
                            outT[p][rows, qi * 512:(qi + 1) * 512],
                            nm_sb[:], rb[:])
                per_p.append((cu, norm))

            # Stitch the p=0 flush/norm units into p=1's first (QK-only)
            # units: p1's exps start while p0's last AVs drain, and p0's
            # norm (which frees the o banks) lands before p1's first AV.
            cu0, n0 = per_p[0]
            cu1, n1 = per_p[1]
            units += cu0[:-AV_LAG]
            units += [cu1[0], cu0[-2], cu1[1], cu0[-1], n0]
            units += cu1[2:]
            units.append(n1)
            for qc in range(4 * qi, 4 * qi + 4):
                def yproj(qc=qc):
                    qcs = slice(qc * 128, (qc + 1) * 128)
                    y_ps = [psP.tile([128, 512], F32, tag="proj",
                                     name=f"y{qc}{ct}") for ct in range(2)]
                    for p in range(2):
                        for ct in range(2):
                            nc.tensor.matmul(
                                y_ps[ct][:], outT[p][:, qcs],
                                wout_t[p][:, ct * 512:(ct + 1) * 512],
                                start=(p == 0), stop=(p == 1))
                    for ct in range(2):
                        y_sb = ysp.tile([128, 512], BF16, tag="ysb",
                                        name=f"ysb{qc}{ct}")
                        nc.vector.tensor_copy(y_sb[:], y_ps[ct][:])
                        q_eng = nc.gpsimd if ct == 0 else nc.sync
                        q_eng.dma_start(y_d[qcs, ct * 512:(ct + 1) * 512],
                                        y_sb[:])
                units.append(yproj)
            return units

        # ---- emit rounds ----
        rounds = [proj_units(0)]
        for qi in range(NSL):
            a = attn_units(qi)
            b = proj_units(qi + 1) if qi + 1 < NSL else []
            rounds.append(_merge(a, b))
        for rr in rounds:
            for u in rr:
                u()

    nc.compile()
    return nc


_NC = None


def _get_nc():
    global _NC
    if _NC is None:
        _NC = _build_nc()
    return _NC


def _host_shards(x, w_qkv, b_qkv, w_out, b_out, pos_bias):
    import ml_dtypes
    x = np.asarray(x, dtype=np.float32)
    w_qkv = np.asarray(w_qkv, dtype=np.float32)
    b_qkv = np.asarray(b_qkv, dtype=np.float32)
    w_out = np.asarray(w_out, dtype=np.float32)
    pos_bias = np.asarray(pos_bias, dtype=np.float32).reshape(HEADS, DH)

    wq, wk, wv = w_qkv[:, :DIM], w_qkv[:, DIM:2 * DIM], w_qkv[:, 2 * DIM:]
    bq, bk, bv = b_qkv[:DIM], b_qkv[DIM:2 * DIM], b_qkv[2 * DIM:]

    dk = np.arange(128)[:, None]
    dq = np.arange(128)[None, :]
    masktri = np.where(dk <= dq, 0.0, -240.0).astype(ml_dtypes.bfloat16)

    maps = []
    for core in range(NCORES):
        b, g = divmod(core, HPC)
        h0 = HPC * g
        cols = slice(h0 * DH, (h0 + HPC) * DH)          # 256 head dims
        pair_cols = [slice((h0 + 2 * p) * DH, (h0 + 2 * p + 2) * DH)
                     for p in range(2)]
        wqk_c = np.concatenate(
            [wq[:, pair_cols[0]], wq[:, pair_cols[1]],
             wk[:, pair_cols[0]], wk[:, pair_cols[1]]], axis=1)
        qbias = np.stack(
            [bq[pair_cols[p]]
             + pos_bias[h0 + 2 * p:h0 + 2 * p + 2].reshape(-1)
             for p in range(2)], axis=1)
        kbias = np.stack([bk[pair_cols[p]] for p in range(2)], axis=1)
        bvb = np.broadcast_to(bv[cols], (128, HPC * DH))
        wout_c = np.stack([w_out[pair_cols[p], :] for p in range(2)])
        maps.append({
            "xt": np.ascontiguousarray(x[b].T).astype(ml_dtypes.bfloat16),
            "wqk": np.ascontiguousarray(wqk_c).astype(ml_dtypes.bfloat16),
            "wv": np.ascontiguousarray(wv[:, cols]).astype(ml_dtypes.bfloat16),
            "qbias": np.ascontiguousarray(qbias),
            "kbias": np.ascontiguousarray(kbias),
            "bvb": np.ascontiguousarray(bvb),
            "wout": np.ascontiguousarray(wout_c).astype(ml_dtypes.bfloat16),
            "masktri": masktri,
        })
    return maps


def kernel(x, w_qkv, b_qkv, w_out, b_out, pos_bias, _trace=False):
    nc = _get_nc()
    in_maps = _host_shards(x, w_qkv, b_qkv, w_out, b_out, pos_bias)
    res = run_bass_kernel_spmd(nc, in_maps, list(range(NCORES)),
                               trace=_trace)
    b_out = np.asarray(b_out, dtype=np.float32)
    y = np.empty((B, T, DIM), dtype=np.float32)
    for b in range(B):
        acc = res.results[b * HPC]["y"].astype(np.float32)
        for g in range(1, HPC):
            acc = acc + res.results[b * HPC + g]["y"].astype(np.float32)
        y[b] = acc + b_out
    if _trace:
        kernel._last_results = res
    return y
